# revision 16
# baseline (speedup 1.0000x reference)
"""Binarized 3-layer MLP (B=8192, H=4096) on 8 Trainium2 NeuronCores.

Strategy: data-parallel over batch (1024 rows/core), weights replicated.
All matmul operands are exactly +-1, so the GEMMs are exact in fp8
(products +-1, fp32 PSUM accumulation of <=4096 terms). BatchNorm+binarize
folds into an integer threshold per output channel: the GEMM output y is an
even integer in [-4096, 4096] and gamma*rsqrt(var+eps) > 0, so
  sign(BN(y)) = +1  <=>  y >= T_o
for an even-integer threshold T_o computed on the host. On-device this is a
single ScalarE Sign activation with per-partition bias 1 - T_o (y + 1 - T_o
is an odd integer, so no 0-boundary ambiguity).

Layout is feature-major throughout: activations live in SBUF as
[128 partitions (h within chunk), 32 chunks x 1024 batch]. The GEMMs run in
fp8e4 with perf_mode=DoubleRow (two fp8 weights per PE cell -> 256-deep
contraction per matmul): each layer is 32 o-tiles x (16 double-chunks x 2
batch-halves) accumulating matmuls (lhsT [128,2,128], rhs [128,2,512])
followed by one ScalarE Sign over the [128, 1024] PSUM tile, written to the
other activation plane. PE streams at its DoubleRow rate (~216 ns per
512-col matmul), so the only recoverable time is at the edges:

- a burst of dependency-free warm-up matmuls on garbage SBUF runs during
  the prologue + input DMA fill, so the HAM clock-gate is at 8/8 before the
  first real matmul and the PE is never idle-cold;
- x arrives as 32 half-tiles round-robined over three DMA queues and the
  first weight tile is split into d-chunk pieces, so the first real matmul
  only waits for ~160 KB, not 768 KB;
- the 10-wide output layer uses 4-way PE column tiling with the LAST eight
  chunks assigned to column group 0; each group's partial sum is DMAed
  straight from PSUM to DRAM when its accumulation finishes (three of the
  four during the output-layer stream), and the host adds the four partials.
  No on-device cross-partition reduce at all.

Measured ~695 us baseline; this variant targets the ~23 us of edge overhead.
"""

import numpy as np
import ml_dtypes

N_CORES = 8
B, H, L, NCOUT = 8192, 4096, 3, 10
BC = B // N_CORES          # batch per core
NT = H // 128              # 32 tiles of 128 along any H axis
BN_EPS = np.float32(1e-5)
TN_EPS = np.float32(1e-4)
HALF = BC // 2             # 512: one PSUM bank of fp32 per matmul
N_DUMMY = 20               # warm-up matmuls covering the HBM-bound input fill

TRACE = False              # test harness may flip this for NTFF profiling
TRACE_DIR = None
LAST_EXEC_NS = None
ND = H // 256              # 16 double-row chunks of 256 along contraction

_BUILD_CACHE = {}


def _split_multi_waits(nc):
    """walrus' CoreV3 codegen rejects instructions carrying more than one
    semaphore wait. Hoist all-but-one wait of any multi-wait instruction
    into standalone NoOps (same engine, placed immediately before)."""
    import bass_rust
    import concourse.mybir as mybir

    n = 0
    for f in nc.m.functions:
        for blk in f.blocks:
            out = []
            changed = False
            for inst in blk.instructions:
                si = inst.sync_info
                if si is not None and si.on_wait and len(si.on_wait) > 1:
                    waits = list(si.on_wait)
                    for w in waits[:-1]:
                        n += 1
                        nop = mybir.InstNoOp(name=f"waitsplit_{n}", ins=[], outs=[])
                        nop.engine = inst.engine
                        nop.sync_info = bass_rust.SyncInfo(on_wait=[w], on_update=[])
                        out.append(nop)
                    inst.sync_info = bass_rust.SyncInfo(
                        on_wait=[waits[-1]], on_update=list(si.on_update or [])
                    )
                    changed = True
                out.append(inst)
            if changed:
                blk.instructions = out
    return nc


def _build():
    if "nc" in _BUILD_CACHE:
        return _BUILD_CACHE["nc"]

    import concourse.bass as bass
    import concourse.mybir as mybir
    from concourse.tile import TileContext

    dt_w = mybir.dt.float8e4
    f32 = mybir.dt.float32

    wout_w = NT * NCOUT
    nc = bass.Bass()
    xin = nc.dram_tensor("x", [ND, 2, 128, BC], dt_w, kind="ExternalInput")
    win = nc.dram_tensor("w", [L, NT, 128, H], dt_w, kind="ExternalInput")
    # first weight tile again, pre-chunked so each quarter is a contiguous
    # 128 KB block (win[0,0] slices would be strided 256 B rows)
    w00in = nc.dram_tensor("w00", [4, 128, 1024], dt_w, kind="ExternalInput")
    biasin = nc.dram_tensor("bias", [128, L * NT], f32, kind="ExternalInput")
    woutin = nc.dram_tensor("wout", [128, wout_w], dt_w, kind="ExternalInput")
    # rows 32g..32g+9 hold column-group g's partial sum; host extracts + adds
    outd = nc.dram_tensor("out", [106, BC], f32, kind="ExternalOutput")

    with TileContext(nc) as tc:
        with (
            tc.tile_pool(name="const", bufs=1) as constp,
            tc.tile_pool(name="acts", bufs=1) as actp,
            tc.tile_pool(name="wpool", bufs=4) as wp,
            tc.tile_pool(name="psum", bufs=4, space="PSUM") as pp,
        ):
            plane0 = actp.tile([128, NT * BC], dt_w, tag="plane0")
            plane1 = actp.tile([128, NT * BC], dt_w, tag="plane1")
            planes = [plane0, plane1]

            # ---- warm-up: dependency-free matmuls on garbage SBUF so the PE
            # is busy during the DMA fill and HAM reaches 8/8 before the real
            # stream. Writes rotate the psum pool's first buffer; nothing
            # reads the result (the pool frees it after the last writer).
            dummy_ps = pp.tile([128, BC], f32, tag="ps", name="dummy_ps")
            for i in range(N_DUMMY):
                nc.tensor.matmul(
                    dummy_ps[:, 0:HALF],
                    plane0[:, 0:128],
                    plane0[:, 0:HALF],
                    start=True,
                    stop=True,
                )

            # layer-1 input: 16 chunk-pair tiles, each DMAed as two
            # contiguous batch-half pieces (the host layout makes each piece
            # exactly one matmul's rhs). The two HWDGE queues carry x only
            # during the fill; the gpsimd SWDGE queue carries the first four
            # weight tiles + BN constants so x gets the HW queues to itself.
            xtiles = [
                actp.tile([128, 2 * BC], dt_w, tag=f"xt{dd}", name=f"xt{dd}")
                for dd in range(ND)
            ]
            # first weight tile in contiguous quarter pieces on sync: piece 0
            # (d-chunks 0-3) unblocks the first matmuls.
            wt00 = wp.tile([128, H], dt_w, tag="wt", name="wt00")
            for q in range(4):
                nc.sync.dma_start(
                    wt00[:, q * 1024 : (q + 1) * 1024], w00in[q]
                )
            bias_t = constp.tile([128, L * NT], f32, tag="bias")
            nc.gpsimd.dma_start(bias_t[:], biasin[:])
            wout_t = constp.tile([128, wout_w], dt_w, tag="wout")
            nc.gpsimd.dma_start(wout_t[:], woutin[:])
            for dd in range(ND):
                eng = nc.scalar if dd % 2 == 0 else nc.sync
                for h in range(2):
                    eng.dma_start(
                        xtiles[dd][:, h * BC : (h + 1) * BC],
                        xin[dd, h],
                    )

            cur = 0
            for l in range(L):
                src, dst = planes[cur], planes[1 - cur]
                src3 = src[:].rearrange("p (c b) -> p c b", c=NT)
                for t in range(NT):
                    if l == 0 and t == 0:
                        wt = wt00
                    else:
                        wt = wp.tile([128, H], dt_w, tag="wt")
                        nc.sync.dma_start(wt[:], win[l, t])
                    ps = pp.tile([128, BC], f32, tag="ps")
                    w3 = wt[:].rearrange("p (d j m) -> p d j m", d=ND, j=2)
                    for d in range(ND):
                        lhsT = w3[:, d]
                        if l == 0:
                            x4 = xtiles[d][:].rearrange(
                                "p (h j b) -> p h j b", h=2, j=2
                            )
                            a0 = x4[:, 0]
                            a1 = x4[:, 1]
                        else:
                            a0 = src3[:, 2 * d : 2 * d + 2, 0:HALF]
                            a1 = src3[:, 2 * d : 2 * d + 2, HALF:BC]
                        nc.tensor.matmul(
                            ps[:, 0:HALF], lhsT, a0,
                            start=(d == 0), stop=(d == ND - 1),
                            perf_mode=mybir.MatmulPerfMode.DoubleRow,
                        )
                        nc.tensor.matmul(
                            ps[:, HALF:BC], lhsT, a1,
                            start=(d == 0), stop=(d == ND - 1),
                            perf_mode=mybir.MatmulPerfMode.DoubleRow,
                        )
                    bias_ap = bias_t[:, l * NT + t : l * NT + t + 1]
                    if l == L - 1 and t == NT - 1:
                        # split the very last Sign so the output layer's
                        # chunk-31 matmuls unblock half a Sign earlier
                        nc.scalar.sign(
                            dst[:, t * BC : t * BC + HALF], ps[:, 0:HALF],
                            bias=bias_ap,
                        )
                        nc.scalar.sign(
                            dst[:, t * BC + HALF : (t + 1) * BC], ps[:, HALF:BC],
                            bias=bias_ap,
                        )
                    else:
                        nc.scalar.sign(dst[:, t * BC : (t + 1) * BC], ps[:], bias=bias_ap)
                cur = 1 - cur

            src = planes[cur]
            # final 10-channel layer: 4-way column tiling -- chunk c runs in
            # column group c%4 (concurrent in the PE array), partial sums
            # land at PSUM partitions 32g..32g+9; the in-between rows are
            # garbage. After the last stop matmul, ONE wide DVE copy per
            # batch half moves rows 0-105 to SBUF (cost is per-column, so
            # copying the garbage rows is free) and a bulk DMA ships them;
            # the host extracts the four 10-row blocks and sums them.
            psf = pp.tile([128, BC], f32, tag="ps", name="psf")
            po = constp.tile([106, BC], f32, tag="po")
            for c in range(NT):
                g = c % 4
                lhsT = wout_t[:, c * NCOUT : (c + 1) * NCOUT]
                a0 = src[:, c * BC : c * BC + HALF]
                a1 = src[:, c * BC + HALF : (c + 1) * BC]
                nc.tensor.matmul(
                    psf[32 * g : 32 * g + NCOUT, 0:HALF], lhsT, a0,
                    start=(c < 4), stop=(c >= NT - 4),
                    tile_position=(0, 32 * g),
                )
                nc.tensor.matmul(
                    psf[32 * g : 32 * g + NCOUT, HALF:BC], lhsT, a1,
                    start=(c < 4), stop=(c >= NT - 4),
                    tile_position=(0, 32 * g),
                )
            for h in range(2):
                nc.vector.tensor_copy(
                    po[:, h * HALF : (h + 1) * HALF],
                    psf[0:106, h * HALF : (h + 1) * HALF],
                )
                nc.sync.dma_start(
                    outd[:, h * HALF : (h + 1) * HALF],
                    po[:, h * HALF : (h + 1) * HALF],
                )

    _split_multi_waits(nc)
    _BUILD_CACHE["nc"] = nc
    return nc


def _thresholds(bn_gamma, bn_beta, bn_mean, bn_var):
    """Per-channel even-integer threshold T with sign(BN(y)) = +1 <=> y >= T,
    mirroring the reference's fp32 arithmetic. gamma>0 so BN is increasing."""
    arg = (bn_var.astype(np.float32) + BN_EPS).astype(np.float32)  # fp32 add as in ref
    rs = (1.0 / np.sqrt(arg.astype(np.float64))).astype(np.float32)
    y = np.arange(-H, H + 1, 2, dtype=np.float32)[:, None]  # [4097, 1]
    T = np.empty((L, H), np.float32)
    for l in range(L):
        z = ((y - bn_mean[l]) * rs[l]) * bn_gamma[l] + bn_beta[l]
        nz = z >= 0
        first = nz.argmax(axis=0)
        anyt = nz.any(axis=0)
        T[l] = np.where(anyt, -H + 2.0 * first, H + 2.0)
    return T


def kernel(x, W, Wout, bn_gamma, bn_beta, bn_mean, bn_var, tn_w, tn_b, tn_m, tn_v):
    global LAST_EXEC_NS
    from concourse.bass_utils import run_bass_kernel_spmd

    x = np.asarray(x, dtype=np.float32)
    W = np.asarray(W, dtype=np.float32)
    Wout = np.asarray(Wout, dtype=np.float32)
    bn_gamma = np.asarray(bn_gamma, dtype=np.float32)
    bn_beta = np.asarray(bn_beta, dtype=np.float32)
    bn_mean = np.asarray(bn_mean, dtype=np.float32)
    bn_var = np.asarray(bn_var, dtype=np.float32)

    np_dt = ml_dtypes.float8_e4m3

    # --- host prep: binarize + lay out ---
    xb = np.where(x.reshape(B, H) >= np.float32(0.5), 1.0, -1.0).astype(np_dt)
    xb = np.ascontiguousarray(xb.T)  # [H, B] feature-major

    Ws = np.where(W >= 0, 1.0, -1.0).astype(np_dt)  # [L, O, H]
    # w_dev[l, t, k, d*256 + j*128 + m] = Ws[l, t*128+m, (2d+j)*128+k]
    w_dev = np.ascontiguousarray(
        Ws.reshape(L, NT, 128, ND, 2, 128)
        .transpose(0, 1, 5, 3, 4, 2)
        .reshape(L, NT, 128, H)
    )

    T = _thresholds(bn_gamma, bn_beta, bn_mean, bn_var)
    # bias[p, l*NT+t] = 1 - T[l, t*128+p]
    bias_host = np.ascontiguousarray(
        (np.float32(1.0) - T).reshape(L, NT, 128).transpose(2, 0, 1).reshape(128, L * NT)
    )

    WoS = np.where(Wout >= 0, 1.0, -1.0).astype(np_dt)  # [10, H]
    # wout[k, c*10+j] = WoS[j, c*128+k]
    wout_host = np.ascontiguousarray(
        WoS.reshape(NCOUT, NT, 128).transpose(2, 1, 0).reshape(128, NT * NCOUT)
    )

    nc = _build()
    # first weight tile pre-chunked into contiguous 128 KB quarters
    w00_host = np.ascontiguousarray(
        w_dev[0, 0].reshape(128, 4, 1024).transpose(1, 0, 2)
    )
    in_maps = []
    for core in range(N_CORES):
        sl = slice(core * BC, (core + 1) * BC)
        # half-major within a pair tile: xc[d, h, p, j*512 + b] =
        # xb[(2d+j)*128 + p, h*512 + b] -- each [128, 1024] half is exactly
        # one DoubleRow matmul's rhs ([128, 2, 512]), contiguous in DRAM.
        xc = np.ascontiguousarray(
            xb[:, sl]
            .reshape(ND, 2, 128, 2, HALF)       # [d, j, p, h, b]
            .transpose(0, 3, 2, 1, 4)           # [d, h, p, j, b]
            .reshape(ND, 2, 128, BC)
        )
        in_maps.append(
            {"x": xc, "w": w_dev, "w00": w00_host,
             "bias": bias_host, "wout": wout_host}
        )

    kwargs = {}
    if TRACE:
        kwargs = {"trace": True, "tmpdir": TRACE_DIR}
    # the first device open occasionally hits a transient
    # NRT_EXEC_UNIT_UNRECOVERABLE (e.g. racing another process's nrt_close);
    # a retry has always recovered it
    import time

    last_exc = None
    for attempt in range(3):
        try:
            res = run_bass_kernel_spmd(nc, in_maps, list(range(N_CORES)), **kwargs)
            break
        except Exception as exc:  # noqa: BLE001
            last_exc = exc
            time.sleep(5 * (attempt + 1))
    else:
        raise last_exc
    LAST_EXEC_NS = res.exec_time_ns

    out_int = np.concatenate(
        [
            sum(
                np.asarray(res.results[c]["out"][32 * g : 32 * g + NCOUT],
                           dtype=np.float32)
                for g in range(4)
            ).T
            for c in range(N_CORES)
        ],
        axis=0,
    )  # [B, 10] exact even integers (sum of 4 exact-int partials)

    rs_t = np.float32(1.0 / np.sqrt(np.float64(np.float32(tn_v) + TN_EPS)))
    out = ((out_int - np.float32(tn_m)) * rs_t) * np.float32(tn_w) + np.float32(tn_b)
    return out.astype(np.float32)


# revision 23
# speedup vs baseline: 1.0191x; 1.0191x over previous
"""Binarized 3-layer MLP (B=8192, H=4096) on 8 Trainium2 NeuronCores.

Strategy: data-parallel over batch (1024 rows/core), weights replicated.
All matmul operands are exactly +-1, so the GEMMs are exact in fp8
(products +-1, fp32 PSUM accumulation of <=4096 terms). BatchNorm+binarize
folds into an integer threshold per output channel: the GEMM output y is an
even integer in [-4096, 4096] and gamma*rsqrt(var+eps) > 0, so
  sign(BN(y)) = +1  <=>  y >= T_o
for an even-integer threshold T_o computed on the host. On-device this is a
single ScalarE Sign activation with per-partition bias 1 - T_o (y + 1 - T_o
is an odd integer, so no 0-boundary ambiguity).

Layout is feature-major throughout: activations live in SBUF as
[128 partitions (h within chunk), 32 chunks x 1024 batch]. The GEMMs run in
fp8e4 with perf_mode=DoubleRow (two fp8 weights per PE cell -> 256-deep
contraction per matmul): each layer is 32 o-tiles x (16 double-chunks x 2
batch-halves) accumulating matmuls (lhsT [128,2,128], rhs [128,2,512])
followed by one ScalarE Sign over the [128, 1024] PSUM tile, written to the
other activation plane. PE streams at its DoubleRow rate (~216 ns per
512-col matmul), so the only recoverable time is at the edges:

- a burst of dependency-free warm-up matmuls on garbage SBUF runs during
  the prologue + input DMA fill, so the HAM clock-gate is at 8/8 before the
  first real matmul and the PE is never idle-cold;
- x arrives as 32 half-tiles round-robined over three DMA queues and the
  first weight tile is split into d-chunk pieces, so the first real matmul
  only waits for ~160 KB, not 768 KB;
- the 10-wide output layer uses 4-way PE column tiling with the LAST eight
  chunks assigned to column group 0; each group's partial sum is DMAed
  straight from PSUM to DRAM when its accumulation finishes (three of the
  four during the output-layer stream), and the host adds the four partials.
  No on-device cross-partition reduce at all.

Measured ~695 us baseline; this variant targets the ~23 us of edge overhead.
"""

import numpy as np
import ml_dtypes

N_CORES = 8
B, H, L, NCOUT = 8192, 4096, 3, 10
BC = B // N_CORES          # batch per core
NT = H // 128              # 32 tiles of 128 along any H axis
BN_EPS = np.float32(1e-5)
TN_EPS = np.float32(1e-4)
HALF = BC // 2             # 512: one PSUM bank of fp32 per matmul
N_DUMMY = 20               # warm-up matmuls covering the HBM-bound input fill

TRACE = False              # test harness may flip this for NTFF profiling
TRACE_DIR = None
LAST_EXEC_NS = None
ND = H // 256              # 16 double-row chunks of 256 along contraction

_BUILD_CACHE = {}


def _split_multi_waits(nc):
    """walrus' CoreV3 codegen rejects instructions carrying more than one
    semaphore wait. Hoist all-but-one wait of any multi-wait instruction
    into standalone NoOps (same engine, placed immediately before)."""
    import bass_rust
    import concourse.mybir as mybir

    n = 0
    for f in nc.m.functions:
        for blk in f.blocks:
            out = []
            changed = False
            for inst in blk.instructions:
                si = inst.sync_info
                if si is not None and si.on_wait and len(si.on_wait) > 1:
                    waits = list(si.on_wait)
                    for w in waits[:-1]:
                        n += 1
                        nop = mybir.InstNoOp(name=f"waitsplit_{n}", ins=[], outs=[])
                        nop.engine = inst.engine
                        nop.sync_info = bass_rust.SyncInfo(on_wait=[w], on_update=[])
                        out.append(nop)
                    inst.sync_info = bass_rust.SyncInfo(
                        on_wait=[waits[-1]], on_update=list(si.on_update or [])
                    )
                    changed = True
                out.append(inst)
            if changed:
                blk.instructions = out
    return nc


def _build():
    if "nc" in _BUILD_CACHE:
        return _BUILD_CACHE["nc"]

    import concourse.bass as bass
    import concourse.mybir as mybir
    from concourse.tile import TileContext

    dt_w = mybir.dt.float8e4
    f32 = mybir.dt.float32

    wout_w = NT * NCOUT
    nc = bass.Bass()
    # x pair-tile 0 as two contiguous 128 KB halves (each half is exactly one
    # matmul's rhs) so the first matmul gates on 128 KB; tiles 1-15 as full
    # contiguous 256 KB tiles (few big DMAs -- the HWDGE lane-completion
    # handshake serializes small-DMA floods).
    x0in = nc.dram_tensor("x0", [2, 128, BC], dt_w, kind="ExternalInput")
    xrin = nc.dram_tensor("xr", [ND - 1, 128, 2 * BC], dt_w, kind="ExternalInput")
    win = nc.dram_tensor("w", [L, NT, 128, H], dt_w, kind="ExternalInput")
    # first weight tile pre-chunked so d-chunk quarters are contiguous
    w00in = nc.dram_tensor("w00", [4, 128, 1024], dt_w, kind="ExternalInput")
    biasin = nc.dram_tensor("bias", [128, L * NT], f32, kind="ExternalInput")
    woutin = nc.dram_tensor("wout", [128, wout_w], dt_w, kind="ExternalInput")
    # column-group g's partial sum; host adds the four
    outd = nc.dram_tensor("out", [4, NCOUT, BC], f32, kind="ExternalOutput")

    with TileContext(nc) as tc:
        with (
            tc.tile_pool(name="const", bufs=1) as constp,
            tc.tile_pool(name="acts", bufs=1) as actp,
            tc.tile_pool(name="wpool", bufs=4) as wp,
            tc.tile_pool(name="psum", bufs=4, space="PSUM") as pp,
        ):
            plane0 = actp.tile([128, NT * BC], dt_w, tag="plane0")
            plane1 = actp.tile([128, NT * BC], dt_w, tag="plane1")
            planes = [plane0, plane1]

            # ---- warm-up: dependency-free matmuls on garbage SBUF so the PE
            # is busy during the DMA fill and HAM reaches 8/8 before the real
            # stream. Writes rotate the psum pool's first buffer; nothing
            # reads the result (the pool frees it after the last writer).
            dummy_ps = pp.tile([128, BC], f32, tag="ps", name="dummy_ps")
            for i in range(N_DUMMY):
                nc.tensor.matmul(
                    dummy_ps[:, 0:HALF],
                    plane0[:, 0:128],
                    plane0[:, 0:HALF],
                    start=True,
                    stop=True,
                )

            # layer-1 input: 16 chunk-pair tiles, each DMAed as two
            # contiguous batch-half pieces (the host layout makes each piece
            # exactly one matmul's rhs). The two HWDGE queues carry x only
            # during the fill; the gpsimd SWDGE queue carries the first four
            # weight tiles + BN constants so x gets the HW queues to itself.
            xtiles = [
                actp.tile([128, 2 * BC], dt_w, tag=f"xt{dd}", name=f"xt{dd}")
                for dd in range(ND)
            ]
            # first weight tile: quarter 0 (d-chunks 0-3, 128 KB) unblocks
            # the first matmuls; the rest in one DMA behind it.
            wt00 = wp.tile([128, H], dt_w, tag="wt", name="wt00")
            nc.sync.dma_start(wt00[:, 0:1024], w00in[0])
            nc.sync.dma_start(wt00[:, 1024:4096], win[0, 0, :, 1024:4096])
            bias_t = constp.tile([128, L * NT], f32, tag="bias")
            nc.gpsimd.dma_start(bias_t[:], biasin[:])
            wout_t = constp.tile([128, wout_w], dt_w, tag="wout")
            nc.gpsimd.dma_start(wout_t[:], woutin[:])
            for h in range(2):
                nc.scalar.dma_start(
                    xtiles[0][:, h * BC : (h + 1) * BC], x0in[h]
                )
            for dd in range(1, ND):
                eng = nc.scalar if dd % 2 == 0 else nc.sync
                eng.dma_start(xtiles[dd][:], xrin[dd - 1])

            cur = 0
            for l in range(L):
                src, dst = planes[cur], planes[1 - cur]
                src3 = src[:].rearrange("p (c b) -> p c b", c=NT)
                for t in range(NT):
                    if l == 0 and t == 0:
                        wt = wt00
                    else:
                        wt = wp.tile([128, H], dt_w, tag="wt")
                        nc.sync.dma_start(wt[:], win[l, t])
                    ps = pp.tile([128, BC], f32, tag="ps")
                    w3 = wt[:].rearrange("p (d j m) -> p d j m", d=ND, j=2)
                    for d in range(ND):
                        lhsT = w3[:, d]
                        if l == 0:
                            x4 = xtiles[d][:].rearrange(
                                "p (h j b) -> p h j b", h=2, j=2
                            )
                            a0 = x4[:, 0]
                            a1 = x4[:, 1]
                        else:
                            a0 = src3[:, 2 * d : 2 * d + 2, 0:HALF]
                            a1 = src3[:, 2 * d : 2 * d + 2, HALF:BC]
                        nc.tensor.matmul(
                            ps[:, 0:HALF], lhsT, a0,
                            start=(d == 0), stop=(d == ND - 1),
                            perf_mode=mybir.MatmulPerfMode.DoubleRow,
                        )
                        nc.tensor.matmul(
                            ps[:, HALF:BC], lhsT, a1,
                            start=(d == 0), stop=(d == ND - 1),
                            perf_mode=mybir.MatmulPerfMode.DoubleRow,
                        )
                    bias_ap = bias_t[:, l * NT + t : l * NT + t + 1]
                    if l == L - 1 and t == NT - 1:
                        # split the very last Sign so the output layer's
                        # chunk-31 matmuls unblock half a Sign earlier
                        nc.scalar.sign(
                            dst[:, t * BC : t * BC + HALF], ps[:, 0:HALF],
                            bias=bias_ap,
                        )
                        nc.scalar.sign(
                            dst[:, t * BC + HALF : (t + 1) * BC], ps[:, HALF:BC],
                            bias=bias_ap,
                        )
                    else:
                        nc.scalar.sign(dst[:, t * BC : (t + 1) * BC], ps[:], bias=bias_ap)
                cur = 1 - cur

            src = planes[cur]
            # final 10-channel layer: 4-way column tiling -- chunk c runs in
            # column group c%4 (concurrent in the PE array). Each group
            # accumulates in its OWN psum tile (separate banks), so the
            # per-group drain copy cannot alias a later group's accumulating
            # writes (bank-granular dependency tracking would serialize
            # them). Group g stops at chunk 28+g; its [10, 1024] partial is
            # copied to SBUF (alternating ScalarE/DVE so two copies run
            # concurrently) and DMAed out; the host adds the four partials.
            fgs = [
                pp.tile([128, BC], f32, tag="ps", name=f"fg{g}") for g in range(4)
            ]
            po = constp.tile([NCOUT, 4 * BC], f32, tag="po")
            for c in range(NT):
                g = c % 4
                lhsT = wout_t[:, c * NCOUT : (c + 1) * NCOUT]
                a0 = src[:, c * BC : c * BC + HALF]
                a1 = src[:, c * BC + HALF : (c + 1) * BC]
                nc.tensor.matmul(
                    fgs[g][32 * g : 32 * g + NCOUT, 0:HALF], lhsT, a0,
                    start=(c < 4), stop=(c >= NT - 4),
                    tile_position=(0, 32 * g),
                )
                nc.tensor.matmul(
                    fgs[g][32 * g : 32 * g + NCOUT, HALF:BC], lhsT, a1,
                    start=(c < 4), stop=(c >= NT - 4),
                    tile_position=(0, 32 * g),
                )
                if c >= NT - 4:
                    pslice = po[:, g * BC : (g + 1) * BC]
                    if g % 2 == 0:
                        nc.scalar.copy(
                            pslice, fgs[g][32 * g : 32 * g + NCOUT, :]
                        )
                    else:
                        nc.vector.tensor_copy(
                            pslice, fgs[g][32 * g : 32 * g + NCOUT, :]
                        )
                    nc.sync.dma_start(outd[g], pslice)

    _split_multi_waits(nc)
    _BUILD_CACHE["nc"] = nc
    return nc


def _thresholds(bn_gamma, bn_beta, bn_mean, bn_var):
    """Per-channel even-integer threshold T with sign(BN(y)) = +1 <=> y >= T,
    mirroring the reference's fp32 arithmetic. gamma>0 so BN is increasing."""
    arg = (bn_var.astype(np.float32) + BN_EPS).astype(np.float32)  # fp32 add as in ref
    rs = (1.0 / np.sqrt(arg.astype(np.float64))).astype(np.float32)
    y = np.arange(-H, H + 1, 2, dtype=np.float32)[:, None]  # [4097, 1]
    T = np.empty((L, H), np.float32)
    for l in range(L):
        z = ((y - bn_mean[l]) * rs[l]) * bn_gamma[l] + bn_beta[l]
        nz = z >= 0
        first = nz.argmax(axis=0)
        anyt = nz.any(axis=0)
        T[l] = np.where(anyt, -H + 2.0 * first, H + 2.0)
    return T


def kernel(x, W, Wout, bn_gamma, bn_beta, bn_mean, bn_var, tn_w, tn_b, tn_m, tn_v):
    global LAST_EXEC_NS
    from concourse.bass_utils import run_bass_kernel_spmd

    x = np.asarray(x, dtype=np.float32)
    W = np.asarray(W, dtype=np.float32)
    Wout = np.asarray(Wout, dtype=np.float32)
    bn_gamma = np.asarray(bn_gamma, dtype=np.float32)
    bn_beta = np.asarray(bn_beta, dtype=np.float32)
    bn_mean = np.asarray(bn_mean, dtype=np.float32)
    bn_var = np.asarray(bn_var, dtype=np.float32)

    np_dt = ml_dtypes.float8_e4m3

    # --- host prep: binarize + lay out ---
    xb = np.where(x.reshape(B, H) >= np.float32(0.5), 1.0, -1.0).astype(np_dt)
    xb = np.ascontiguousarray(xb.T)  # [H, B] feature-major

    Ws = np.where(W >= 0, 1.0, -1.0).astype(np_dt)  # [L, O, H]
    # w_dev[l, t, k, d*256 + j*128 + m] = Ws[l, t*128+m, (2d+j)*128+k]
    w_dev = np.ascontiguousarray(
        Ws.reshape(L, NT, 128, ND, 2, 128)
        .transpose(0, 1, 5, 3, 4, 2)
        .reshape(L, NT, 128, H)
    )

    T = _thresholds(bn_gamma, bn_beta, bn_mean, bn_var)
    # bias[p, l*NT+t] = 1 - T[l, t*128+p]
    bias_host = np.ascontiguousarray(
        (np.float32(1.0) - T).reshape(L, NT, 128).transpose(2, 0, 1).reshape(128, L * NT)
    )

    WoS = np.where(Wout >= 0, 1.0, -1.0).astype(np_dt)  # [10, H]
    # wout[k, c*10+j] = WoS[j, c*128+k]
    wout_host = np.ascontiguousarray(
        WoS.reshape(NCOUT, NT, 128).transpose(2, 1, 0).reshape(128, NT * NCOUT)
    )

    nc = _build()
    # first weight tile pre-chunked into contiguous 128 KB quarters
    w00_host = np.ascontiguousarray(
        w_dev[0, 0].reshape(128, 4, 1024).transpose(1, 0, 2)
    )
    in_maps = []
    for core in range(N_CORES):
        sl = slice(core * BC, (core + 1) * BC)
        # half-major within a pair tile: xc[d, p, h*512*2...] with layout
        # [p, (h j b)] so each [*, 1024] half is one DoubleRow rhs. Tile 0
        # ships as two contiguous halves [h, p, (j b)]; tiles 1-15 as full
        # contiguous [p, (h j b)] tiles.
        xc = np.ascontiguousarray(
            xb[:, sl]
            .reshape(ND, 2, 128, 2, HALF)       # [d, j, p, h, b]
            .transpose(0, 2, 3, 1, 4)           # [d, p, h, j, b]
            .reshape(ND, 128, 2 * BC)
        )
        x0 = np.ascontiguousarray(
            xc[0].reshape(128, 2, BC).transpose(1, 0, 2)
        )
        in_maps.append(
            {"x0": x0, "xr": xc[1:], "w": w_dev, "w00": w00_host,
             "bias": bias_host, "wout": wout_host}
        )

    kwargs = {}
    if TRACE:
        kwargs = {"trace": True, "tmpdir": TRACE_DIR}
    # the first device open occasionally hits a transient
    # NRT_EXEC_UNIT_UNRECOVERABLE (e.g. racing another process's nrt_close);
    # a retry has always recovered it
    import time

    last_exc = None
    for attempt in range(3):
        try:
            res = run_bass_kernel_spmd(nc, in_maps, list(range(N_CORES)), **kwargs)
            break
        except Exception as exc:  # noqa: BLE001
            last_exc = exc
            time.sleep(5 * (attempt + 1))
    else:
        raise last_exc
    LAST_EXEC_NS = res.exec_time_ns

    out_int = np.concatenate(
        [
            np.asarray(res.results[c]["out"], dtype=np.float32).sum(axis=0).T
            for c in range(N_CORES)
        ],
        axis=0,
    )  # [B, 10] exact even integers (sum of 4 exact-int partials)

    rs_t = np.float32(1.0 / np.sqrt(np.float64(np.float32(tn_v) + TN_EPS)))
    out = ((out_int - np.float32(tn_m)) * rs_t) * np.float32(tn_w) + np.float32(tn_b)
    return out.astype(np.float32)


# revision 30
# speedup vs baseline: 1.0203x; 1.0012x over previous
"""Binarized 3-layer MLP (B=8192, H=4096) on 8 Trainium2 NeuronCores.

Strategy: data-parallel over batch (1024 rows/core), weights replicated.
All matmul operands are exactly +-1, so the GEMMs are exact in fp8
(products +-1, fp32 PSUM accumulation of <=4096 terms). BatchNorm+binarize
folds into an integer threshold per output channel: the GEMM output y is an
even integer in [-4096, 4096] and gamma*rsqrt(var+eps) > 0, so
  sign(BN(y)) = +1  <=>  y >= T_o
for an even-integer threshold T_o computed on the host. On-device this is a
single ScalarE Sign activation with per-partition bias 1 - T_o (y + 1 - T_o
is an odd integer, so no 0-boundary ambiguity).

Layout is feature-major throughout: activations live in SBUF as
[128 partitions (h within chunk), 32 chunks x 1024 batch]. The GEMMs run in
fp8e4 with perf_mode=DoubleRow (two fp8 weights per PE cell -> 256-deep
contraction per matmul): each layer is 32 o-tiles x (16 double-chunks x 2
batch-halves) accumulating matmuls (lhsT [128,2,128], rhs [128,2,512])
followed by one ScalarE Sign over the [128, 1024] PSUM tile, written to the
other activation plane. PE streams at its DoubleRow rate (~216 ns per
512-col matmul), so the only recoverable time is at the edges:

- a burst of dependency-free warm-up matmuls on garbage SBUF runs during
  the prologue + input DMA fill, so the HAM clock-gate is at 8/8 before the
  first real matmul and the PE is never idle-cold;
- x arrives as 32 half-tiles round-robined over three DMA queues and the
  first weight tile is split into d-chunk pieces, so the first real matmul
  only waits for ~160 KB, not 768 KB;
- the 10-wide output layer uses 4-way PE column tiling with the LAST eight
  chunks assigned to column group 0; each group's partial sum is DMAed
  straight from PSUM to DRAM when its accumulation finishes (three of the
  four during the output-layer stream), and the host adds the four partials.
  No on-device cross-partition reduce at all.

Measured ~695 us baseline; this variant targets the ~23 us of edge overhead.
"""

import numpy as np
import ml_dtypes

N_CORES = 8
B, H, L, NCOUT = 8192, 4096, 3, 10
BC = B // N_CORES          # batch per core
NT = H // 128              # 32 tiles of 128 along any H axis
BN_EPS = np.float32(1e-5)
TN_EPS = np.float32(1e-4)
HALF = BC // 2             # 512: one PSUM bank of fp32 per matmul
N_DUMMY = 6                # warm-up matmuls bridging prologue -> first data

TRACE = False              # test harness may flip this for NTFF profiling
TRACE_DIR = None
LAST_EXEC_NS = None
ND = H // 256              # 16 double-row chunks of 256 along contraction

_BUILD_CACHE = {}


def _split_multi_waits(nc):
    """walrus' CoreV3 codegen rejects instructions carrying more than one
    semaphore wait. Hoist all-but-one wait of any multi-wait instruction
    into standalone NoOps (same engine, placed immediately before)."""
    import bass_rust
    import concourse.mybir as mybir

    n = 0
    for f in nc.m.functions:
        for blk in f.blocks:
            out = []
            changed = False
            for inst in blk.instructions:
                si = inst.sync_info
                if si is not None and si.on_wait and len(si.on_wait) > 1:
                    waits = list(si.on_wait)
                    for w in waits[:-1]:
                        n += 1
                        nop = mybir.InstNoOp(name=f"waitsplit_{n}", ins=[], outs=[])
                        nop.engine = inst.engine
                        nop.sync_info = bass_rust.SyncInfo(on_wait=[w], on_update=[])
                        out.append(nop)
                    inst.sync_info = bass_rust.SyncInfo(
                        on_wait=[waits[-1]], on_update=list(si.on_update or [])
                    )
                    changed = True
                out.append(inst)
            if changed:
                blk.instructions = out
    return nc


def _build():
    if "nc" in _BUILD_CACHE:
        return _BUILD_CACHE["nc"]

    import concourse.bass as bass
    import concourse.mybir as mybir
    from concourse.tile import TileContext

    dt_w = mybir.dt.float8e4
    f32 = mybir.dt.float32

    wout_w = NT * NCOUT
    nc = bass.Bass()
    # x pair-tile 0 as two contiguous 128 KB halves (each half is exactly one
    # matmul's rhs) so the first matmul gates on 128 KB; tiles 1-15 as full
    # contiguous 256 KB tiles (few big DMAs -- the HWDGE lane-completion
    # handshake serializes small-DMA floods).
    x0in = nc.dram_tensor("x0", [2, 128, BC], dt_w, kind="ExternalInput")
    xrin = nc.dram_tensor("xr", [ND - 1, 128, 2 * BC], dt_w, kind="ExternalInput")
    win = nc.dram_tensor("w", [L, NT, 128, H], dt_w, kind="ExternalInput")
    # layer-1 o-tiles 0-3 repacked d-major ([d, t, j, m] per partition k) so
    # the first four o-tiles can accumulate TOGETHER, d-round by d-round, as
    # x pairs arrive -- each pair feeds 8 matmuls, matching DMA delivery.
    # Shipped as four contiguous 512 KB quarters (d-chunks 4q..4q+3).
    wblkin = nc.dram_tensor("wblk", [4, 128, 4 * 1024], dt_w, kind="ExternalInput")
    biasin = nc.dram_tensor("bias", [128, L * NT], f32, kind="ExternalInput")
    woutin = nc.dram_tensor("wout", [128, wout_w], dt_w, kind="ExternalInput")
    # column-group g's partial sum; host adds the four
    outd = nc.dram_tensor("out", [4, NCOUT, BC], f32, kind="ExternalOutput")

    with TileContext(nc) as tc:
        with (
            tc.tile_pool(name="const", bufs=1) as constp,
            tc.tile_pool(name="acts", bufs=1) as actp,
            tc.tile_pool(name="wpool", bufs=4) as wp,
            tc.tile_pool(name="psum", bufs=4, space="PSUM") as pp,
        ):
            plane0 = actp.tile([128, NT * BC], dt_w, tag="plane0")
            plane1 = actp.tile([128, NT * BC], dt_w, tag="plane1")
            planes = [plane0, plane1]

            # ---- warm-up: dependency-free matmuls on garbage SBUF so the PE
            # is busy during the DMA fill and HAM reaches 8/8 before the real
            # stream. Writes rotate the psum pool's first buffer; nothing
            # reads the result (the pool frees it after the last writer).
            dummy_ps = pp.tile([128, BC], f32, tag="ps", name="dummy_ps")
            for i in range(N_DUMMY):
                nc.tensor.matmul(
                    dummy_ps[:, 0:HALF],
                    plane0[:, 0:128],
                    plane0[:, 0:HALF],
                    start=True,
                    stop=True,
                )

            # layer-1 input: 16 chunk-pair tiles, each DMAed as two
            # contiguous batch-half pieces (the host layout makes each piece
            # exactly one matmul's rhs). The two HWDGE queues carry x only
            # during the fill; the gpsimd SWDGE queue carries the first four
            # weight tiles + BN constants so x gets the HW queues to itself.
            xtiles = [
                actp.tile([128, 2 * BC], dt_w, tag=f"xt{dd}", name=f"xt{dd}")
                for dd in range(ND)
            ]
            # block weights (layer-1 o-tiles 0-3, d-major) in four contiguous
            # 512 KB quarters on sync; all x rides scalar so the two streams
            # flow concurrently at queue rate.
            wblk_t = actp.tile([128, ND * 4 * 256], dt_w, tag="wblk")
            for q in range(4):
                nc.sync.dma_start(
                    wblk_t[:, q * 4096 : (q + 1) * 4096], wblkin[q]
                )
            bias_t = constp.tile([128, L * NT], f32, tag="bias")
            nc.gpsimd.dma_start(bias_t[:], biasin[:])
            wout_t = constp.tile([128, wout_w], dt_w, tag="wout")
            nc.gpsimd.dma_start(wout_t[:], woutin[:])
            for h in range(2):
                nc.scalar.dma_start(
                    xtiles[0][:, h * BC : (h + 1) * BC], x0in[h]
                )
            for dd in range(1, ND):
                nc.scalar.dma_start(xtiles[dd][:], xrin[dd - 1])

            def sign_tile(l, t, dst, ps, split):
                bias_ap = bias_t[:, l * NT + t : l * NT + t + 1]
                if split:
                    # split the very last Sign so the output layer's
                    # chunk-31 matmuls unblock half a Sign earlier
                    nc.scalar.sign(
                        dst[:, t * BC : t * BC + HALF], ps[:, 0:HALF],
                        bias=bias_ap,
                    )
                    nc.scalar.sign(
                        dst[:, t * BC + HALF : (t + 1) * BC], ps[:, HALF:BC],
                        bias=bias_ap,
                    )
                else:
                    nc.scalar.sign(dst[:, t * BC : (t + 1) * BC], ps[:], bias=bias_ap)

            # ---- layer 1, o-tiles 0-3: d-round-robin across the four tiles
            # so each x pair is consumed four times on arrival
            blk_ps = [
                pp.tile([128, BC], f32, tag="ps", name=f"blk{t}") for t in range(4)
            ]
            wb4 = wblk_t[:].rearrange("p (d t j m) -> p d t j m", d=ND, t=4, j=2)
            for d in range(ND):
                x4 = xtiles[d][:].rearrange("p (h j b) -> p h j b", h=2, j=2)
                for t in range(4):
                    lhsT = wb4[:, d, t]
                    nc.tensor.matmul(
                        blk_ps[t][:, 0:HALF], lhsT, x4[:, 0],
                        start=(d == 0), stop=(d == ND - 1),
                        perf_mode=mybir.MatmulPerfMode.DoubleRow,
                    )
                    nc.tensor.matmul(
                        blk_ps[t][:, HALF:BC], lhsT, x4[:, 1],
                        start=(d == 0), stop=(d == ND - 1),
                        perf_mode=mybir.MatmulPerfMode.DoubleRow,
                    )
            for t in range(4):
                sign_tile(0, t, planes[1], blk_ps[t], False)

            cur = 0
            for l in range(L):
                src, dst = planes[cur], planes[1 - cur]
                src3 = src[:].rearrange("p (c b) -> p c b", c=NT)
                for t in range(4 if l == 0 else 0, NT):
                    wt = wp.tile([128, H], dt_w, tag="wt")
                    nc.sync.dma_start(wt[:], win[l, t])
                    ps = pp.tile([128, BC], f32, tag="ps")
                    w3 = wt[:].rearrange("p (d j m) -> p d j m", d=ND, j=2)
                    for d in range(ND):
                        lhsT = w3[:, d]
                        if l == 0:
                            x4 = xtiles[d][:].rearrange(
                                "p (h j b) -> p h j b", h=2, j=2
                            )
                            a0 = x4[:, 0]
                            a1 = x4[:, 1]
                        else:
                            a0 = src3[:, 2 * d : 2 * d + 2, 0:HALF]
                            a1 = src3[:, 2 * d : 2 * d + 2, HALF:BC]
                        nc.tensor.matmul(
                            ps[:, 0:HALF], lhsT, a0,
                            start=(d == 0), stop=(d == ND - 1),
                            perf_mode=mybir.MatmulPerfMode.DoubleRow,
                        )
                        nc.tensor.matmul(
                            ps[:, HALF:BC], lhsT, a1,
                            start=(d == 0), stop=(d == ND - 1),
                            perf_mode=mybir.MatmulPerfMode.DoubleRow,
                        )
                    sign_tile(l, t, dst, ps, l == L - 1 and t == NT - 1)
                cur = 1 - cur

            src = planes[cur]
            # final 10-channel layer: 4-way column tiling -- chunk c runs in
            # column group c%4 (concurrent in the PE array). Each group
            # accumulates in its OWN psum tile (separate banks), so the
            # per-group drain copy cannot alias a later group's accumulating
            # writes (bank-granular dependency tracking would serialize
            # them). Group g stops at chunk 28+g; its [10, 1024] partial is
            # copied to SBUF (alternating ScalarE/DVE so two copies run
            # concurrently) and DMAed out; the host adds the four partials.
            fgs = [
                pp.tile([128, BC], f32, tag="ps", name=f"fg{g}") for g in range(4)
            ]
            po = constp.tile([NCOUT, 4 * BC], f32, tag="po")
            for c in range(NT):
                g = c % 4
                lhsT = wout_t[:, c * NCOUT : (c + 1) * NCOUT]
                a0 = src[:, c * BC : c * BC + HALF]
                a1 = src[:, c * BC + HALF : (c + 1) * BC]
                nc.tensor.matmul(
                    fgs[g][32 * g : 32 * g + NCOUT, 0:HALF], lhsT, a0,
                    start=(c < 4), stop=(c >= NT - 4),
                    tile_position=(0, 32 * g),
                )
                nc.tensor.matmul(
                    fgs[g][32 * g : 32 * g + NCOUT, HALF:BC], lhsT, a1,
                    start=(c < 4), stop=(c >= NT - 4),
                    tile_position=(0, 32 * g),
                )
                if c >= NT - 4:
                    pslice = po[:, g * BC : (g + 1) * BC]
                    if g % 2 == 0:
                        nc.scalar.copy(
                            pslice, fgs[g][32 * g : 32 * g + NCOUT, :]
                        )
                        nc.sync.dma_start(outd[g], pslice)
                    else:
                        nc.vector.tensor_copy(
                            pslice, fgs[g][32 * g : 32 * g + NCOUT, :]
                        )
                        nc.scalar.dma_start(outd[g], pslice)

    _split_multi_waits(nc)
    _BUILD_CACHE["nc"] = nc
    return nc


def _thresholds(bn_gamma, bn_beta, bn_mean, bn_var):
    """Per-channel even-integer threshold T with sign(BN(y)) = +1 <=> y >= T,
    mirroring the reference's fp32 arithmetic. gamma>0 so BN is increasing."""
    arg = (bn_var.astype(np.float32) + BN_EPS).astype(np.float32)  # fp32 add as in ref
    rs = (1.0 / np.sqrt(arg.astype(np.float64))).astype(np.float32)
    y = np.arange(-H, H + 1, 2, dtype=np.float32)[:, None]  # [4097, 1]
    T = np.empty((L, H), np.float32)
    for l in range(L):
        z = ((y - bn_mean[l]) * rs[l]) * bn_gamma[l] + bn_beta[l]
        nz = z >= 0
        first = nz.argmax(axis=0)
        anyt = nz.any(axis=0)
        T[l] = np.where(anyt, -H + 2.0 * first, H + 2.0)
    return T


def kernel(x, W, Wout, bn_gamma, bn_beta, bn_mean, bn_var, tn_w, tn_b, tn_m, tn_v):
    global LAST_EXEC_NS
    from concourse.bass_utils import run_bass_kernel_spmd

    x = np.asarray(x, dtype=np.float32)
    W = np.asarray(W, dtype=np.float32)
    Wout = np.asarray(Wout, dtype=np.float32)
    bn_gamma = np.asarray(bn_gamma, dtype=np.float32)
    bn_beta = np.asarray(bn_beta, dtype=np.float32)
    bn_mean = np.asarray(bn_mean, dtype=np.float32)
    bn_var = np.asarray(bn_var, dtype=np.float32)

    np_dt = ml_dtypes.float8_e4m3

    # --- host prep: binarize + lay out ---
    xb = np.where(x.reshape(B, H) >= np.float32(0.5), 1.0, -1.0).astype(np_dt)
    xb = np.ascontiguousarray(xb.T)  # [H, B] feature-major

    Ws = np.where(W >= 0, 1.0, -1.0).astype(np_dt)  # [L, O, H]
    # w_dev[l, t, k, d*256 + j*128 + m] = Ws[l, t*128+m, (2d+j)*128+k]
    w_dev = np.ascontiguousarray(
        Ws.reshape(L, NT, 128, ND, 2, 128)
        .transpose(0, 1, 5, 3, 4, 2)
        .reshape(L, NT, 128, H)
    )

    T = _thresholds(bn_gamma, bn_beta, bn_mean, bn_var)
    # bias[p, l*NT+t] = 1 - T[l, t*128+p]
    bias_host = np.ascontiguousarray(
        (np.float32(1.0) - T).reshape(L, NT, 128).transpose(2, 0, 1).reshape(128, L * NT)
    )

    WoS = np.where(Wout >= 0, 1.0, -1.0).astype(np_dt)  # [10, H]
    # wout[k, c*10+j] = WoS[j, c*128+k]
    wout_host = np.ascontiguousarray(
        WoS.reshape(NCOUT, NT, 128).transpose(2, 1, 0).reshape(128, NT * NCOUT)
    )

    nc = _build()
    # layer-1 o-tiles 0-3 repacked d-major: wblk[d, k, t, j*128+m] =
    # Ws[0, t*128+m, (2d+j)*128+k], shipped as four contiguous quarters
    # wblk3[q, k, dd*1024 + t*256 + j*128 + m] with d = 4q+dd.
    Ws0 = np.where(W[0, :512] >= 0, 1.0, -1.0).astype(np_dt)  # [512, H]
    wblk2 = (
        Ws0.reshape(4, 128, ND, 2, 128)       # [t, m, d, j, k]
        .transpose(2, 4, 0, 3, 1)             # [d, k, t, j, m]
        .reshape(ND, 128, 1024)
    )
    wblk3 = np.ascontiguousarray(
        wblk2.reshape(4, 4, 128, 1024)        # [q, dd, k, t*256+jm]
        .transpose(0, 2, 1, 3)                # [q, k, dd, ...]
        .reshape(4, 128, 4096)
    )
    in_maps = []
    for core in range(N_CORES):
        sl = slice(core * BC, (core + 1) * BC)
        # half-major within a pair tile: xc[d, p, h*512*2...] with layout
        # [p, (h j b)] so each [*, 1024] half is one DoubleRow rhs. Tile 0
        # ships as two contiguous halves [h, p, (j b)]; tiles 1-15 as full
        # contiguous [p, (h j b)] tiles.
        xc = np.ascontiguousarray(
            xb[:, sl]
            .reshape(ND, 2, 128, 2, HALF)       # [d, j, p, h, b]
            .transpose(0, 2, 3, 1, 4)           # [d, p, h, j, b]
            .reshape(ND, 128, 2 * BC)
        )
        x0 = np.ascontiguousarray(
            xc[0].reshape(128, 2, BC).transpose(1, 0, 2)
        )
        in_maps.append(
            {"x0": x0, "xr": xc[1:], "w": w_dev, "wblk": wblk3,
             "bias": bias_host, "wout": wout_host}
        )

    kwargs = {}
    if TRACE:
        kwargs = {"trace": True, "tmpdir": TRACE_DIR}
    # the first device open occasionally hits a transient
    # NRT_EXEC_UNIT_UNRECOVERABLE (e.g. racing another process's nrt_close);
    # a retry has always recovered it
    import time

    last_exc = None
    for attempt in range(3):
        try:
            res = run_bass_kernel_spmd(nc, in_maps, list(range(N_CORES)), **kwargs)
            break
        except Exception as exc:  # noqa: BLE001
            last_exc = exc
            time.sleep(5 * (attempt + 1))
    else:
        raise last_exc
    LAST_EXEC_NS = res.exec_time_ns

    out_int = np.concatenate(
        [
            np.asarray(res.results[c]["out"], dtype=np.float32).sum(axis=0).T
            for c in range(N_CORES)
        ],
        axis=0,
    )  # [B, 10] exact even integers (sum of 4 exact-int partials)

    rs_t = np.float32(1.0 / np.sqrt(np.float64(np.float32(tn_v) + TN_EPS)))
    out = ((out_int - np.float32(tn_m)) * rs_t) * np.float32(tn_w) + np.float32(tn_b)
    return out.astype(np.float32)
